# revision 11
# baseline (speedup 1.0000x reference)
"""Multi-head causal self-attention (B=2, T=2048, C=1024, H=16, D=64) on 8
Trainium2 NeuronCores.

Sharding: data-parallel over batch (2) x tensor-parallel over heads (4 groups
of 4 heads) = 8 shards, no cross-core communication. Host sums the 4 partial
outputs per batch and adds the (folded) bias.

All matmuls in bf16 (1 PE cycle/row at any moving size; fp8/DoubleRow was
evaluated but every fp8 station exceeds the 2e-2 accuracy gate). Per core:
  qkT = wqk.T @ xT            [4x128, T]   (chunks: q01 q23 k01 k23)
  v   = xT.T @ wv             [T, 4, 64]+ones col (rhs layout for PV)
  per (tk 128-key block, head): scT = k_blk.T @ qT -> exp -> pt [keys, queries]
  PV non-transposed: att[q, h, 0:65] += pt[:, qblk].T @ v[:, tk, h, 0:65]
     (col 64 = ones -> per-query softmax denominator lands per PSUM partition,
      so normalization is one reciprocal + one broadcast multiply on the DVE)
  attT via PE transpose (identity matmul; the DMA XBAR route has ~2.5us
     chain latency that stalled the projection)
  out = attT.T @ wproj  -> bf16 out DMA; host adds b_proj + bv@wproj.

The attention inner loop is latency-bound (exp on the scalar engine), so the
qkv projection and output projection are split into filler units appended
AFTER each attention block's steps: the tile scheduler pops ready work by
emission-index priority, so attention (which feeds the saturated Act engine)
always wins ties while fillers absorb every PE stall. qkv(t) fills block
t-1; all projection work fills the final (most exp-bound) block. The causal
mask is a DVE multiply with a precomputed triangle; all bias matmuls are
folded into the host epilogue (softmax rows sum to 1, so the V bias
contributes bv @ w_proj to every output row).
"""

import numpy as np
import ml_dtypes

import concourse.bass as bass
import concourse.mybir as mybir
import concourse.tile as tile
from concourse import bacc
from concourse.bass_utils import run_bass_kernel_spmd

f32 = mybir.dt.float32
bf16 = mybir.dt.bfloat16
AF = mybir.ActivationFunctionType
ALU = mybir.AluOpType

B, T, C, H, D = 2, 2048, 1024, 16, 64
HPC = 4          # heads per core
NCORES = 8
TQ = 512         # query tile of the attention outer loop
NTQ = T // TQ    # 4
NKC = C // 128   # 8 contraction chunks for the qkv projection
NTT = T // 128   # 16 query 128-blocks
SCALE = 1.0 / 8.0  # 1/sqrt(D)

_CACHE = {}


def build_nc():
    nc = bacc.Bacc("TRN2", target_bir_lowering=False, debug=False)

    xt_d = nc.dram_tensor("xt", [C, T], bf16, kind="ExternalInput")
    wqkv_d = nc.dram_tensor("wqkv", [C, 768], bf16, kind="ExternalInput")
    bqk_d = nc.dram_tensor("bqk", [128, 4], f32, kind="ExternalInput")
    wproj_d = nc.dram_tensor("wproj", [256, C], bf16, kind="ExternalInput")
    out_d = nc.dram_tensor("out", [T, C], bf16, kind="ExternalOutput")

    with tile.TileContext(nc) as tc:
        with (
            tc.tile_pool(name="const", bufs=1) as const,
            tc.tile_pool(name="xts", bufs=3) as xts_pool,
            tc.tile_pool(name="pt", bufs=28) as pt_pool,
            tc.tile_pool(name="atts", bufs=8) as atts_pool,
            tc.tile_pool(name="rec", bufs=8) as rec_pool,
            tc.tile_pool(name="ot", bufs=8) as ot_pool,
            tc.tile_pool(name="ps_mm", bufs=2, space="PSUM") as ps_mm,
            tc.tile_pool(name="ps_sc", bufs=2, space="PSUM") as ps_sc,
            tc.tile_pool(name="ps_att", bufs=4, space="PSUM") as ps_att,
        ):
            # ---- resident tensors; DMAs chunked so compute starts early ----
            wqkv_sb = const.tile([128, NKC, 768], bf16, tag="wqkv")
            wqkv_r = wqkv_d.rearrange("(o p) n -> p o n", p=128)
            nc.scalar.dma_start(wqkv_sb[:, 0, :], wqkv_r[:, 0, :])
            nc.scalar.dma_start(wqkv_sb[:, 1:4, :], wqkv_r[:, 1:4, :])
            nc.scalar.dma_start(wqkv_sb[:, 4:NKC, :], wqkv_r[:, 4:NKC, :])
            bqk_sb = const.tile([128, 4], f32, tag="bqk")
            nc.scalar.dma_start(bqk_sb[:], bqk_d[:, :])
            wproj_sb = const.tile([128, 2, C], bf16, tag="wproj")
            nc.scalar.dma_start(wproj_sb[:], wproj_d.rearrange("(o p) n -> p o n", p=128))

            # qkT chunks: 0 = qT heads01, 1 = qT heads23, 2 = kT h01, 3 = kT h23
            qkT_sb = const.tile([128, 4, T], bf16, tag="qkT")
            # v in PV-rhs layout: [key mod 128, key block, head, 64 vdims + one]
            v_sb = const.tile([128, NTT, HPC, 65], bf16, tag="v")
            nc.vector.memset(v_sb[:, :, :, 64:65], 1.0)
            # attT: chunk hc: partitions = head-dims of heads (2hc, 2hc+1)
            attT_sb = const.tile([128, 2, T], bf16, tag="attT")
            # identity for PE-transpose of the normalized attention
            ident_sb = const.tile([128, 128], bf16, tag="ident")
            nc.vector.memset(ident_sb[:], 1.0)
            nc.gpsimd.affine_select(
                ident_sb[:],
                ident_sb[:],
                pattern=[[1, 128]],
                compare_op=ALU.is_equal,
                fill=0.0,
                base=0,
                channel_multiplier=-1,
            )
            # lower-triangular causal mask (keep j >= p), applied to diagonal
            # blocks with a DVE multiply (lower latency than gpsimd select)
            tri_sb = const.tile([128, 128], bf16, tag="tri")
            nc.vector.memset(tri_sb[:], 1.0)
            nc.gpsimd.affine_select(
                tri_sb[:],
                tri_sb[:],
                pattern=[[1, 128]],
                compare_op=ALU.is_ge,
                fill=0.0,
                base=0,
                channel_multiplier=-1,
            )

            xt_r = xt_d.rearrange("(o p) t -> p o t", p=128)

            def emit_qkv(tq):
                tqs = slice(TQ * tq, TQ * (tq + 1))
                xts = xts_pool.tile([128, NKC, TQ], bf16, tag="xts")
                nc.sync.dma_start(xts[:, 0, :], xt_r[:, 0, tqs])
                nc.sync.dma_start(xts[:, 1:4, :], xt_r[:, 1:4, tqs])
                nc.sync.dma_start(xts[:, 4:NKC, :], xt_r[:, 4:NKC, tqs])
                # q,k transposed: psum = wqkv_chunk.T @ xT
                if tq == 0:
                    # kc-major over pairs of open psum groups to hide DMA ramp
                    for cps in ((0, 2), (1, 3)):
                        ps_pair = [
                            ps_mm.tile([128, TQ], f32, tag="mm", name=f"qk{cp}")
                            for cp in cps
                        ]
                        for kc in range(NKC):
                            for i, cp in enumerate(cps):
                                nc.tensor.matmul(
                                    ps_pair[i][:],
                                    lhsT=wqkv_sb[:, kc, 128 * cp : 128 * (cp + 1)],
                                    rhs=xts[:, kc, :],
                                    start=(kc == 0),
                                    stop=(kc == NKC - 1),
                                    skip_group_check=True,
                                )
                        for i, cp in enumerate(cps):
                            nc.vector.tensor_scalar_add(
                                qkT_sb[:, cp, tqs], ps_pair[i][:], bqk_sb[:, cp : cp + 1]
                            )
                else:
                    for cp in range(4):
                        ps = ps_mm.tile([128, TQ], f32, tag="mm")
                        for kc in range(NKC):
                            nc.tensor.matmul(
                                ps[:],
                                lhsT=wqkv_sb[:, kc, 128 * cp : 128 * (cp + 1)],
                                rhs=xts[:, kc, :],
                                start=(kc == 0),
                                stop=(kc == NKC - 1),
                            )
                        nc.vector.tensor_scalar_add(
                            qkT_sb[:, cp, tqs], ps[:], bqk_sb[:, cp : cp + 1]
                        )
                # v: psum = xT_chunk.T @ wv  (no bias: folded into host output)
                for tt in range(4 * tq, 4 * tq + 4):
                    psv = ps_mm.tile([128, TQ], f32, tag="mm")
                    toff = 128 * tt - TQ * tq
                    for kc in range(NKC):
                        nc.tensor.matmul(
                            psv[:, 0:256],
                            lhsT=xts[:, kc, toff : toff + 128],
                            rhs=wqkv_sb[:, kc, 512:768],
                            start=(kc == 0),
                            stop=(kc == NKC - 1),
                        )
                    # [128, 256] psum -> [128, 4, 64] slot of v_sb (head-strided)
                    nc.vector.tensor_copy(v_sb[:, tt, :, 0:64], psv[:, 0:256])

            def qkv_units(tq):
                """qkv projection for t-slice tq as filler closures (one psum
                group each) interleaved into the attention instruction stream
                so the PE has independent work while Act runs exp."""
                tqs = slice(TQ * tq, TQ * (tq + 1))
                xts = xts_pool.tile([128, NKC, TQ], bf16, tag="xts")

                def load():
                    nc.sync.dma_start(xts[:, 0:4, :], xt_r[:, 0:4, tqs])
                    nc.sync.dma_start(xts[:, 4:NKC, :], xt_r[:, 4:NKC, tqs])

                def qk_unit(cp):
                    def emit():
                        ps = ps_mm.tile([128, TQ], f32, tag="mm")
                        for kc in range(NKC):
                            nc.tensor.matmul(
                                ps[:],
                                lhsT=wqkv_sb[:, kc, 128 * cp : 128 * (cp + 1)],
                                rhs=xts[:, kc, :],
                                start=(kc == 0),
                                stop=(kc == NKC - 1),
                            )
                        nc.vector.tensor_scalar_add(
                            qkT_sb[:, cp, tqs], ps[:], bqk_sb[:, cp : cp + 1]
                        )
                    return emit

                def v_unit(tt):
                    def emit():
                        psv = ps_mm.tile([128, TQ], f32, tag="mm")
                        toff = 128 * tt - TQ * tq
                        for kc in range(NKC):
                            nc.tensor.matmul(
                                psv[:, 0:256],
                                lhsT=xts[:, kc, toff : toff + 128],
                                rhs=wqkv_sb[:, kc, 512:768],
                                start=(kc == 0),
                                stop=(kc == NKC - 1),
                            )
                        nc.vector.tensor_copy(v_sb[:, tt, :, 0:64], psv[:, 0:256])
                    return emit

                return load, [qk_unit(cp) for cp in range(4)] + [
                    v_unit(tt) for tt in range(4 * tq, 4 * tq + 4)
                ]

            def proj_units_qb(tq, qb):
                tt = 4 * tq + qb
                ot = [None]

                def emit(nt):
                    ts_ = slice(128 * tt, 128 * (tt + 1))
                    ns = slice(512 * nt, 512 * (nt + 1))
                    pso = ps_mm.tile([128, TQ], f32, tag="mm")
                    for hc in range(2):
                        nc.tensor.matmul(
                            pso[:],
                            lhsT=attT_sb[:, hc, ts_],
                            rhs=wproj_sb[:, hc, ns],
                            start=(hc == 0),
                            stop=(hc == 1),
                        )
                    if nt == 0:
                        ot[0] = ot_pool.tile(
                            [128, 2, TQ], bf16, tag="ot", name=f"ot{tt}"
                        )
                    if tq == NTQ - 1:
                        # last block: Act is done with exps by now while the
                        # DVE still drains normalize chains - use Act for the
                        # tail's psum copies, and ship each half as soon as
                        # its copy lands (HWDGE is idle at the end; the
                        # merged DMA would serialize the terminal chain)
                        nc.scalar.copy(ot[0][:, nt, :], pso[:])
                        nc.sync.dma_start(out_d[ts_, ns], ot[0][:, nt, :])
                    else:
                        nc.vector.tensor_copy(ot[0][:, nt, :], pso[:])
                        if nt == 1:
                            # one merged DMA per 128-row block (fewer DMAs =
                            # less serialization on the single-slot HWDGE)
                            nc.sync.dma_start(out_d[ts_, :], ot[0][:])

                return [lambda: emit(0), lambda: emit(1)]

            def proj_units(tq):
                units = []
                for qb in range(4):
                    units.extend(proj_units_qb(tq, qb))
                return units

            def emit_att(tq, fillers, last=False):
                """Attention for tq with PV lagging scores by one step and
                filler matmul units spliced between, so the PE never idles on
                the exp (Act) latency. Each query block's normalize/transpose
                chain is emitted as soon as its diagonal block completes; on
                the last tq the projection units are appended to the filler
                queue the same way, collapsing the pipeline tail."""
                ntk = 4 * tq + 4
                attps = [
                    ps_att.tile([128, HPC, 65], f32, tag="att", name=f"att{tq}_{qb}")
                    for qb in range(4)
                ]
                if tq == 0:
                    steps = [(tk, h) for hp in range(2) for tk in range(ntk)
                             for h in (2 * hp, 2 * hp + 1)]
                else:
                    steps = [(tk, h) for tk in range(ntk) for h in range(HPC)]
                pts = {}

                def emit_sc(i):
                    tk, h = steps[i]
                    d = tk - 4 * tq
                    q0 = 128 * d if d >= 0 else 0
                    w = TQ - q0
                    ks = slice(128 * tk, 128 * (tk + 1))
                    qs = slice(TQ * tq + q0, TQ * (tq + 1))
                    qc, kc_, pr = h // 2, 2 + h // 2, 64 * (h % 2)
                    sc = ps_sc.tile([128, TQ], f32, tag="sc")
                    nc.tensor.matmul(
                        sc[:, 0:w],
                        lhsT=qkT_sb[pr : pr + 64, kc_, ks],
                        rhs=qkT_sb[pr : pr + 64, qc, qs],
                    )
                    pt = pt_pool.tile([128, TQ], bf16, tag="pt")
                    nc.scalar.activation(pt[:, 0:w], sc[:, 0:w], AF.Exp, scale=SCALE)
                    if d >= 0:
                        nc.vector.tensor_mul(pt[:, 0:128], pt[:, 0:128], tri_sb[:])
                    pts[i] = pt

                def emit_pv(i):
                    tk, h = steps[i]
                    d = tk - 4 * tq
                    q0 = 128 * d if d >= 0 else 0
                    pt = pts.pop(i)
                    for qb in range(max(d, 0), 4):
                        qoff = 128 * qb - q0
                        nc.tensor.matmul(
                            attps[qb][:, h, 0:65],
                            lhsT=pt[:, qoff : qoff + 128],
                            rhs=v_sb[:, tk, h, 0:65],
                            start=(tk == 0 and h == 0),
                            stop=(tk == 4 * tq + qb and h == HPC - 1),
                            skip_group_check=True,
                        )

                def emit_norm(qb):
                    rec = rec_pool.tile([128, HPC], f32, tag="rec")
                    nc.vector.reciprocal_approx_fast(
                        out=rec[:], in_=attps[qb][:, :, 64]
                    )
                    att_sb = atts_pool.tile([128, HPC, 64], bf16, tag="atts")
                    nc.vector.tensor_mul(
                        att_sb[:],
                        attps[qb][:, :, 0:64],
                        rec[:, :, None].broadcast_to([128, HPC, 64]),
                    )
                    # transpose on the PE (att_sb [q, hd] -> attT [hd, q]):
                    # ~53ns each vs ~2.5us latency for the DMA XBAR route
                    qslice = slice(TQ * tq + 128 * qb, TQ * tq + 128 * (qb + 1))
                    attTps = ps_att.tile([128, 2, 128], bf16, tag="att")
                    for hc in range(2):
                        nc.tensor.matmul(
                            attTps[:, hc, :],
                            lhsT=att_sb[:, 2 * hc : 2 * hc + 2, :],
                            rhs=ident_sb[:],
                            is_transpose=True,
                            start=(hc == 0),
                            stop=(hc == 1),
                            skip_group_check=True,
                        )
                    nc.vector.tensor_copy(attT_sb[:, :, qslice], attTps[:])

                # PV lags scores by LAG steps so the exp(Act) + mask(DVE)
                # latency is hidden behind later scores/filler matmuls.
                # Dynamically appended fillers (last-tq proj units) are held
                # for DELAY steps: the normalize->transpose->proj readiness
                # chain is long, so scheduling them early just stalls the PE.
                # attention steps first (lowest priority index, so the
                # list scheduler never starves the Act engine), all filler
                # units after (they run whenever the PE would otherwise
                # stall, picked by readiness)
                LAG = 6
                ns = len(steps)
                dyn = []
                for i in range(ns + LAG):
                    if i < ns:
                        emit_sc(i)
                    j = i - LAG
                    if j >= 0:
                        emit_pv(j)
                        tk, h = steps[j]
                        if h == HPC - 1 and tk - 4 * tq >= 0:
                            qb = tk - 4 * tq
                            emit_norm(qb)
                            if last:
                                dyn.extend(proj_units_qb(tq, qb))
                for f in fillers:
                    f()
                for f in dyn:
                    f()

            # software pipeline: qkv(0) as a prologue; the per-tq attention
            # streams carry the remaining qkv/proj matmuls as fillers,
            # distributed by each attention block's Act-vs-PE deficit (the
            # later blocks are increasingly exp-bound, so all proj work is
            # pushed toward them; qkv(t) must complete before att(t) starts).
            emit_qkv(0)
            loads = {}
            plan = {0: [], 1: [], 2: [], 3: []}
            for t in (1, 2, 3):
                load, units = qkv_units(t)
                loads[t - 1] = load
                plan[t - 1] += units
            plan[3] += proj_units(0) + proj_units(1) + proj_units(2)
            for tq in range(NTQ):
                if tq in loads:
                    loads[tq]()
                emit_att(tq, plan[tq], last=(tq == NTQ - 1))

    nc.compile()
    return nc


def _shard_inputs(x, w_qkv, b_qkv, w_proj, b_proj):
    """Full inputs -> per-core input maps. Core c = (batch b=c//4, group g=c%4)."""
    in_maps = []
    xts = [np.ascontiguousarray(x[b].T).astype(ml_dtypes.bfloat16) for b in range(B)]
    for core in range(NCORES):
        b, g = divmod(core, 4)
        qs = slice(256 * g, 256 * (g + 1))
        ks = slice(C + 256 * g, C + 256 * (g + 1))
        vs = slice(2 * C + 256 * g, 2 * C + 256 * (g + 1))
        wqkv = np.concatenate(
            [w_qkv[:, qs], w_qkv[:, ks], w_qkv[:, vs]], axis=1
        ).astype(ml_dtypes.bfloat16)
        bq, bk = b_qkv[qs], b_qkv[ks]
        bqk = np.ascontiguousarray(
            np.stack([bq[0:128], bq[128:256], bk[0:128], bk[128:256]], axis=1)
        ).astype(np.float32)
        wproj = np.ascontiguousarray(w_proj[256 * g : 256 * (g + 1), :]).astype(
            ml_dtypes.bfloat16
        )
        in_maps.append(
            {"xt": xts[b], "wqkv": np.ascontiguousarray(wqkv), "bqk": bqk,
             "wproj": wproj}
        )
    return in_maps


def kernel(x, w_qkv, b_qkv, w_proj, b_proj):
    x = np.asarray(x, dtype=np.float32)
    w_qkv = np.asarray(w_qkv, dtype=np.float32)
    b_qkv = np.asarray(b_qkv, dtype=np.float32)
    w_proj = np.asarray(w_proj, dtype=np.float32)
    b_proj = np.asarray(b_proj, dtype=np.float32)

    if "nc" not in _CACHE:
        _CACHE["nc"] = build_nc()
    nc = _CACHE["nc"]

    in_maps = _shard_inputs(x, w_qkv, b_qkv, w_proj, b_proj)
    res = run_bass_kernel_spmd(nc, in_maps, list(range(NCORES)))
    # host epilogue: sum head-group partials, add folded bias
    b_eff = (b_qkv[2 * C :].astype(np.float64) @ w_proj.astype(np.float64)
             + b_proj).astype(np.float32)
    out = np.empty((B, T, C), dtype=np.float32)
    for b in range(B):
        acc = res.results[4 * b]["out"].astype(np.float32)
        for g in range(1, 4):
            acc = acc + res.results[4 * b + g]["out"].astype(np.float32)
        out[b] = acc + b_eff
    return out
